# revision 19
# baseline (speedup 1.0000x reference)
"""BiMambaVision Trainium2 kernel (8 NeuronCores, SPMD).

Distribution: core c = s*4 + b (s: 0=g-stream, 1=r-stream; b: sample 0..3).
  CNN + ResBlock + co-Mamba fully local per (stream, sample); En phase is
  pairwise tensor-parallel over d_inner between cores (0,b),(1,b):
  AllGather x -> en-res (out-channel shard, identity skip = own x1) ->
  AllGather -> en-mamba with d_inner shard (AllReduce x_proj partials and
  out_proj partials).
Precision: bf16 weights/activations for GEMMs, fp32 PSUM, fp32 dt/dA/scan
  carry (tensor_tensor_scan keeps fp32 state internally).
"""

import os
import numpy as np
import ml_dtypes

BF = ml_dtypes.bfloat16
DEBUG = int(os.environ.get("BMV_DEBUG", "0"))

B, T = 4, 256
FCH = 64
NFCH = T // FCH
FSUBS = [7] * 9 + [1]

_CACHE = {}


def _build():
    import concourse.bass as bass
    import concourse.mybir as mybir
    import concourse.tile as tile
    from concourse import bacc

    F32 = mybir.dt.float32
    BF16 = mybir.dt.bfloat16
    AF = mybir.ActivationFunctionType
    ALU = mybir.AluOpType

    nc = bacc.Bacc("TRN2", target_bir_lowering=False, debug=False, num_devices=8)

    def din(name, shape, dt=BF16):
        return nc.dram_tensor(name, shape, dt, kind="ExternalInput")

    gimc = din("gimc", [100, T * 72])
    w1T = din("w1T", [100, 64])
    w2T = din("w2T", [64, 9 * 128])
    w3T = din("w3T", [128, 9 * 2 * 128])
    w4T = din("w4T", [128, 2 * 64])
    fcT = din("fcT", [128, 8 * 4 * 128])
    cnnb = din("cnnb", [128, 9], F32)
    onesb = din("onesb", [128, 1])
    onesf = din("onesf", [128, 1], F32)
    ecT = din("ecT", [128, 4 * 3 * 8 * 128])
    esT = din("esT", [128, 4 * 8 * 128])
    ebnsb = din("ebnsb", [128, 8 * 2], F32)
    ipT = din("ipT", [128, 8 * 32 * 128])
    cocw = din("cocw", [128, 16 * 4], F32)
    coxpT = din("coxpT", [128, 16 * 96])
    codtT = din("codtT", [64, 16 * 128])
    coopT = din("coopT", [128, 16 * 8 * 128])
    coA = din("coA", [128, 16 * 16], F32)
    cocons = din("cocons", [128, 64], F32)
    enT = din("enT", [128, 16 * 3 * 8 * 128])
    enbnsb = din("enbnsb", [128, 8 * 2], F32)
    eipT = din("eipT", [128, 16 * 32 * 128])
    encw = din("encw", [128, 16 * 4], F32)
    expT = din("expT", [128, 16 * 2 * 128])
    edtT0 = din("edtT0", [128, 16 * 128])
    eopT = din("eopT", [128, 16 * 16 * 128])
    enA = din("enA", [128, 16 * 16], F32)
    encons = din("encons", [128, 80], F32)
    clsT = din("clsT", [128, 16 * 7])
    clsb = din("clsb", [7, 1], F32)

    logits = nc.dram_tensor("logits", [7, 1], F32, kind="ExternalOutput")
    dbg = {}
    if DEBUG:
        for nm, shp, dt in [
            ("d_feat", [128, 4 * 258], BF16),
            ("d_g1", [128, 8 * 256], BF16),
            ("d_xcp", [128, 16 * 256], BF16),
            ("d_dt", [128, 16 * 256], BF16),
            ("d_dbc", [128, 256], BF16),
            ("d_y2", [128, 16 * 256], BF16),
            ("d_oco", [128, 8 * 256], BF16),
            ("d_x1", [128, 8 * 256], BF16),
            ("d_xe", [128, 16 * 256], BF16),
            ("d_excp", [128, 16 * 256], BF16),
            ("d_edt", [128, 16 * 256], BF16),
            ("d_ey2", [128, 16 * 256], BF16),
            ("d_eo", [128, 16 * 256], BF16),
            ("d_xf", [128, 16 * 256], BF16),
        ]:
            dbg[nm] = nc.dram_tensor(nm, shp, dt, kind="ExternalOutput")

    RG = [[i, i + 4] for i in range(4)]

    with tile.TileContext(nc) as tc:
        ctxs = {}

        def pool(name, bufs, space="SBUF"):
            p = tc.tile_pool(name=name, bufs=bufs, space=space)
            ctxs[name] = p
            return p.__enter__()

        def close(*names):
            for nm in names:
                ctxs.pop(nm).__exit__(None, None, None)

        carry = pool("carry", 1)
        dram = pool("dram", 1, "DRAM")
        psum = pool("psum", 4, "PSUM")  # tag "mm": [<=128,256] 4 banks
        psumB = pool("psumB", 3, "PSUM")  # tag "big": [<=128,512] 3 banks
        consts = pool("consts", 1)

        cnnb_t = carry.tile([128, 9], F32)
        nc.sync.dma_start(out=cnnb_t, in_=cnnb[:])
        onesb_t = consts.tile([128, 1], BF16)
        nc.sync.dma_start(out=onesb_t, in_=onesb[:])
        onesf_t = consts.tile([128, 1], F32)
        nc.sync.dma_start(out=onesf_t, in_=onesf[:])

        # ======================= PHASE A: CNN =======================
        carryB = pool("carryB", 1)
        carryA = pool("carryA", 1)
        featpad = carryA.tile([128, 4, 258], BF16)
        nc.vector.memset(featpad, 0.0)
        pooledT = carryA.tile([128, 8, 256], BF16)

        cnnw = pool("cnnw", 1)
        w1t = cnnw.tile([100, 64], BF16)
        nc.sync.dma_start(out=w1t, in_=w1T[:])
        w2t = cnnw.tile([64, 9, 128], BF16)
        nc.sync.dma_start(out=w2t, in_=w2T[:].rearrange("k (t m) -> k t m", t=9))
        w3t = cnnw.tile([128, 9, 2, 128], BF16)
        nc.sync.dma_start(
            out=w3t, in_=w3T[:].rearrange("k (t c m) -> k t c m", t=9, c=2)
        )
        w4t = cnnw.tile([128, 2, 64], BF16)
        nc.sync.dma_start(out=w4t, in_=w4T[:].rearrange("k (c m) -> k c m", c=2))
        fct = cnnw.tile([128, 8, 4, 128], BF16)
        nc.sync.dma_start(
            out=fct, in_=fcT[:].rearrange("k (a b m) -> k a b m", a=8, b=4)
        )

        cnnp = pool("cnnp", 2)
        cnnq = pool("cnnq", 1)
        pads = pool("pads", 1)
        c2pad = [pads.tile([64, FCH, 10, 11], BF16, tag=f"c2p{i}", name=f"c2p{i}") for i in range(2)]
        c3pad = [pads.tile([128, FCH, 10, 11], BF16, tag=f"c3p{i}", name=f"c3p{i}") for i in range(2)]
        for t_ in c2pad + c3pad:
            nc.vector.memset(t_, 0.0)

        gim = cnnw.tile([100, T * 72], BF16)
        nc.sync.dma_start(out=gim, in_=gimc[:])

        for fci in range(NFCH):
            f0 = fci * FCH
            p2, p3 = c2pad[fci % 2], c3pad[fci % 2]
            imcf = gim[:, f0 * 72 : (f0 + FCH) * 72]
            fo = 0
            for fs in FSUBS:
                ps1 = psumB.tile([64, 512], F32, tag="big")
                nc.tensor.matmul(
                    ps1[:, : fs * 72], w1t, imcf[:, fo * 72 : (fo + fs) * 72],
                    start=True, stop=True,
                )
                nc.scalar.activation(
                    p2[:, fo : fo + fs, 1:9, 1:10],
                    ps1[:, : fs * 72].rearrange("p (f h w) -> p f h w", h=8, w=9),
                    AF.Relu,
                    bias=cnnb_t[0:64, 0:1],
                )
                fo += fs
            fo = 0
            for fs in FSUBS:
                ps2 = psumB.tile([128, 512], F32, tag="big")
                pv = ps2[:, : fs * 72].rearrange("p (f h w) -> p f h w", h=8, w=9)
                for tap in range(9):
                    dy, dx = tap // 3, tap % 3
                    nc.tensor.matmul(
                        pv, w2t[:, tap, :],
                        p2[:, fo : fo + fs, dy : dy + 8, dx : dx + 9],
                        start=(tap == 0), stop=(tap == 8),
                    )
                nc.scalar.activation(
                    p3[:, fo : fo + fs, 1:9, 1:10], pv, AF.Relu,
                    bias=cnnb_t[:, 1:2],
                )
                fo += fs
            c4in = cnnq.tile([128, 2, FCH * 72], BF16, tag="c4in")
            fo = 0
            for fs in FSUBS:
                for mc in range(2):
                    ps3 = psumB.tile([128, 512], F32, tag="big")
                    pv = ps3[:, : fs * 72].rearrange("p (f h w) -> p f h w", h=8, w=9)
                    for tap in range(9):
                        dy, dx = tap // 3, tap % 3
                        nc.tensor.matmul(
                            pv, w3t[:, tap, mc, :],
                            p3[:, fo : fo + fs, dy : dy + 8, dx : dx + 9],
                            start=(tap == 0), stop=(tap == 8),
                        )
                    nc.scalar.activation(
                        c4in[:, mc, fo * 72 : (fo + fs) * 72],
                        ps3[:, : fs * 72], AF.Relu,
                        bias=cnnb_t[:, 2 + mc : 3 + mc],
                    )
                fo += fs
            poolin = cnnq.tile([64, FCH * 72], BF16, tag="poolin")
            for nci in range(9):
                ps4 = psumB.tile([64, 512], F32, tag="big")
                for kc in range(2):
                    nc.tensor.matmul(
                        ps4, w4t[:, kc, :],
                        c4in[:, kc, 512 * nci : 512 * (nci + 1)],
                        start=(kc == 0), stop=(kc == 1),
                    )
                nc.scalar.activation(
                    poolin[:, 512 * nci : 512 * (nci + 1)], ps4, AF.Relu,
                    bias=cnnb_t[0:64, 4:5],
                )
            pw = poolin.rearrange("p (f h w) -> p f h w", h=8, w=9)[:, :, :, 0:8]
            pq = pw.rearrange("p f (i a) (j b) -> p f i a j b", a=2, b=2)
            pooled = cnnp.tile([64, FCH, 4, 4], BF16, tag="pooled")
            tmpm = cnnp.tile([64, FCH, 4, 4], BF16, tag="tmpm")
            nc.vector.tensor_tensor(
                out=tmpm, in0=pq[:, :, :, 0, :, 0], in1=pq[:, :, :, 0, :, 1],
                op=ALU.max,
            )
            nc.vector.tensor_tensor(
                out=pooled, in0=pq[:, :, :, 1, :, 0], in1=pq[:, :, :, 1, :, 1],
                op=ALU.max,
            )
            pooledX = cnnp.tile([64, 16, FCH], BF16, tag="pooledX")
            pv_out = pooledX.rearrange("p ij f -> p f ij").rearrange(
                "p f (i j) -> p f i j", i=4, j=4
            )
            nc.vector.tensor_tensor(out=pv_out, in0=pooled, in1=tmpm, op=ALU.max)
            for kc in range(8):
                nc.sync.dma_start(
                    out=pooledT[:, kc, f0 : f0 + FCH],
                    in_=pooledX[kc * 8 : kc * 8 + 8],
                )

        for mc in range(4):
            psf = psum.tile([128, 256], F32, tag="mm")
            for kc in range(8):
                nc.tensor.matmul(
                    psf, fct[:, kc, mc, :], pooledT[:, kc, :],
                    start=(kc == 0), stop=(kc == 7),
                )
            nc.scalar.activation(
                featpad[:, mc, 1:257], psf, AF.Relu,
                bias=cnnb_t[:, 5 + mc : 6 + mc],
            )
        if DEBUG:
            nc.sync.dma_start(
                out=dbg["d_feat"][:], in_=featpad.rearrange("p a t -> p (a t)")
            )

        close("pads", "cnnq", "cnnp", "cnnw")

        # ======================= PHASE B: ResBlock =======================
        ebn_t = carryB.tile([128, 8, 2], F32)
        nc.sync.dma_start(out=ebn_t, in_=ebnsb[:].rearrange("p (a b) -> p a b", b=2))
        g1 = carryB.tile([128, 8, 256], BF16)
        resw = pool("resw", 1)
        ect = resw.tile([128, 4, 3, 8, 128], BF16)
        nc.sync.dma_start(
            out=ect, in_=ecT[:].rearrange("p (k t m c) -> p k t m c", k=4, t=3, m=8)
        )
        est = resw.tile([128, 4, 8, 128], BF16)
        nc.sync.dma_start(
            out=est, in_=esT[:].rearrange("p (k m c) -> p k m c", k=4, m=8)
        )
        resq = pool("resq", 2)
        for mc in range(8):
            psc = psum.tile([128, 256], F32, tag="mm")
            for tap in range(3):
                for kc in range(4):
                    nc.tensor.matmul(
                        psc, ect[:, kc, tap, mc, :],
                        featpad[:, kc, tap : tap + 256],
                        start=(tap == 0 and kc == 0),
                        stop=(tap == 2 and kc == 3),
                    )
            pss = psum.tile([128, 256], F32, tag="mm")
            for kc in range(4):
                nc.tensor.matmul(
                    pss, est[:, kc, mc, :], featpad[:, kc, 1:257],
                    start=(kc == 0), stop=(kc == 3),
                )
            tsb = resq.tile([128, 256], F32, tag="tsb")
            nc.scalar.activation(
                tsb, psc, AF.Relu, bias=ebn_t[:, mc, 1:2], scale=ebn_t[:, mc, 0:1]
            )
            nc.vector.tensor_tensor(out=g1[:, mc, :], in0=tsb, in1=pss, op=ALU.add)
        if DEBUG:
            nc.sync.dma_start(
                out=dbg["d_g1"][:], in_=g1.rearrange("p a t -> p (a t)")
            )

        # =================== mamba block emitter ===================
        def mamba(pref, nkc, x_in, ipT_d, cw_d, cons_d, xpT_d, dtT_d, A_d,
                  opT_d, out_mc, lnslots, resid, x_out, en_mode):
            mp = pool(pref + "_mp", 2)
            mq = pool(pref + "_mq", 1)
            mw = pool(pref + "_mw", 3)
            ndc = 16

            cw_t = mq.tile([128, ndc, 4], F32, tag="cw")
            nc.sync.dma_start(out=cw_t, in_=cw_d[:].rearrange("p (a b) -> p a b", b=4))
            cons_t = mq.tile([128, cons_d.shape[1]], F32, tag="cons")
            nc.sync.dma_start(out=cons_t, in_=cons_d[:])
            A_t = mq.tile([128, ndc, 16], F32, tag="A")
            nc.sync.dma_start(out=A_t, in_=A_d[:].rearrange("p (a b) -> p a b", b=16))

            xcpad = mq.tile([128, ndc, 259], BF16, tag="xcpad")
            nc.vector.memset(xcpad[:, :, 0:3], 0.0)
            zs = mq.tile([128, ndc, 256], BF16, tag="zs")
            ipv = ipT_d[:].rearrange("p (k m c) -> p k m c", k=nkc, m=32)
            for mg in range(8):
                pst = [psum.tile([128, 256], F32, tag="mm", name=f"pst{_i}") for _i in range(4)]
                for kc in range(nkc):
                    wti = mw.tile([128, 4, 128], BF16, tag="wti")
                    nc.sync.dma_start(out=wti, in_=ipv[:, kc, 4 * mg : 4 * mg + 4, :])
                    for i in range(4):
                        nc.tensor.matmul(
                            pst[i], wti[:, i, :], x_in[:, kc, :],
                            start=(kc == 0), stop=(kc == nkc - 1),
                        )
                for i in range(4):
                    mc = 4 * mg + i
                    if mc < 16:
                        nc.scalar.activation(xcpad[:, mc, 3:259], pst[i], AF.Copy)
                    else:
                        nc.scalar.activation(zs[:, mc - 16, :], pst[i], AF.Silu)

            xcp = mq.tile([128, ndc, 256], BF16, tag="xcp")
            for dc in range(ndc):
                dtmp = mq.tile([128, 256], F32, tag="dtmp")
                nc.vector.tensor_scalar_mul(
                    dtmp, xcpad[:, dc, 0:256], cw_t[:, dc, 0:1]
                )
                for j in range(1, 4):
                    nc.vector.scalar_tensor_tensor(
                        out=dtmp, in0=xcpad[:, dc, j : j + 256],
                        scalar=cw_t[:, dc, j : j + 1], in1=dtmp,
                        op0=ALU.mult, op1=ALU.add,
                    )
                nc.scalar.activation(
                    xcp[:, dc, :], dtmp, AF.Silu, bias=cons_t[:, dc : dc + 1]
                )

            if not en_mode:
                xpw = mq.tile([128, ndc, 96], BF16, tag="xpw")
                nc.sync.dma_start(
                    out=xpw, in_=xpT_d[:].rearrange("p (k m) -> p k m", k=ndc)
                )
                psx = psum.tile([96, 256], F32, tag="mm")
                for kc in range(ndc):
                    nc.tensor.matmul(
                        psx, xpw[:, kc, :], xcp[:, kc, :],
                        start=(kc == 0), stop=(kc == ndc - 1),
                    )
                dbc = mq.tile([96, 256], BF16, tag="dbc")
                nc.scalar.activation(dbc, psx, AF.Copy)
                dt_rhs0 = dbc[0:64, :]
                Brow, Crow = dbc[64:80, :], dbc[80:96, :]
            else:
                xpv_d = xpT_d[:].rearrange("p (k c m) -> p k c m", k=ndc, c=2)
                psx0 = psum.tile([128, 256], F32, tag="mm")
                psx1 = psum.tile([128, 256], F32, tag="mm")
                for kc in range(ndc):
                    xpk = mw.tile([128, 2, 128], BF16, tag="xpk")
                    nc.sync.dma_start(out=xpk, in_=xpv_d[:, kc])
                    nc.tensor.matmul(
                        psx0, xpk[:, 0, :], xcp[:, kc, :],
                        start=(kc == 0), stop=(kc == ndc - 1),
                    )
                    nc.tensor.matmul(
                        psx1, xpk[:, 1, :], xcp[:, kc, :],
                        start=(kc == 0), stop=(kc == ndc - 1),
                    )
                dbcp = mq.tile([128, 2, 256], F32, tag="dbcp")
                nc.scalar.activation(dbcp[:, 0, :], psx0, AF.Copy)
                nc.scalar.activation(dbcp[:, 1, :], psx1, AF.Copy)
                arin = dram.tile([128, 2 * 256], F32, tag="arin")
                arout = dram.tile([128, 2 * 256], F32, tag="arout")
                nc.sync.dma_start(out=arin, in_=dbcp.rearrange("p a t -> p (a t)"))
                nc.gpsimd.collective_compute(
                    "AllReduce", ALU.add, replica_groups=RG,
                    ins=[arin.opt()], outs=[arout.opt()],
                )
                dbcf = dbcp
                nc.sync.dma_start(
                    out=dbcf, in_=arout[:].rearrange("p (a t) -> p a t", a=2)
                )
                dbc0b = mq.tile([128, 256], BF16, tag="dbc0b")
                nc.vector.tensor_copy(dbc0b, dbcf[:, 0, :])
                bcs = mq.tile([32, 256], BF16, tag="bcs")
                nc.vector.tensor_copy(bcs, dbcf[0:32, 1, :])
                dt_rhs0 = dbc0b
                Brow, Crow = bcs[0:16, :], bcs[16:32, :]

            dtw = mq.tile([dtT_d.shape[0], ndc, 128], BF16, tag="dtw")
            nc.sync.dma_start(
                out=dtw, in_=dtT_d[:].rearrange("p (m c) -> p m c", m=ndc)
            )
            dtv = dtw
            # softplus(z) = relu(z) + 2*artanh(w/(2+w)), w = exp(-|z|)
            zab = mq.tile([128, ndc, 256], BF16, tag="zab")
            zr = mq.tile([128, ndc, 256], BF16, tag="zr")
            wex = mq.tile([128, ndc, 256], BF16, tag="wex")
            for mc in range(ndc):
                psd = psum.tile([128, 256], F32, tag="mm")
                nc.tensor.matmul(psd, dtv[:, mc, :], dt_rhs0, start=True, stop=True)
                nc.scalar.activation(
                    zab[:, mc, :], psd, AF.Abs, bias=cons_t[:, 16 + mc : 17 + mc]
                )
                nc.scalar.activation(
                    zr[:, mc, :], psd, AF.Relu, bias=cons_t[:, 16 + mc : 17 + mc]
                )
                nc.scalar.activation(wex[:, mc, :], zab[:, mc, :], AF.Exp, scale=-1.0)
            dt_sb = mq.tile([128, ndc, 256], BF16, tag="dtsb")
            for gq in range(4):
                sl = slice(4 * gq, 4 * gq + 4)
                wg = wex[:, sl].rearrange("p a t -> p (a t)")
                den = mq.tile([128, 1024], F32, tag="spden")
                nc.vector.tensor_scalar_add(den, wg, 2.0)
                xa = mq.tile([128, 1024], F32, tag="spxa")
                nc.vector.tensor_scalar(xa, den, -1.0 / 6, 5.0 / 6, ALU.mult, ALU.add)
                er = mq.tile([128, 1024], F32, tag="sper")
                for _nr in range(2):
                    nc.vector.tensor_tensor(out=er, in0=den, in1=xa, op=ALU.mult)
                    nc.vector.tensor_scalar(er, er, -1.0, 2.0, ALU.mult, ALU.add)
                    nc.vector.tensor_tensor(out=xa, in0=xa, in1=er, op=ALU.mult)
                y_ = mq.tile([128, 1024], BF16, tag="spy")
                nc.vector.tensor_tensor(out=y_, in0=wg, in1=xa, op=ALU.mult)
                p_ = mq.tile([128, 1024], BF16, tag="spp")
                nc.vector.tensor_tensor(out=p_, in0=y_, in1=y_, op=ALU.mult)
                v_ = mq.tile([128, 1024], BF16, tag="spv")
                nc.vector.tensor_scalar(v_, p_, 0.2, 1.0 / 3, ALU.mult, ALU.add)
                nc.vector.tensor_tensor(out=p_, in0=y_, in1=p_, op=ALU.mult)
                nc.vector.tensor_tensor(out=p_, in0=p_, in1=v_, op=ALU.mult)
                t3 = mq.tile([128, 1024], BF16, tag="spt3")
                nc.vector.scalar_tensor_tensor(
                    out=t3, in0=y_, scalar=2.0,
                    in1=zr[:, sl].rearrange("p a t -> p (a t)"),
                    op0=ALU.mult, op1=ALU.add,
                )
                nc.vector.scalar_tensor_tensor(
                    out=dt_sb[:, sl].rearrange("p a t -> p (a t)"),
                    in0=p_, scalar=2.0, in1=t3, op0=ALU.mult, op1=ALU.add,
                )

            bcd = dram.tile([1, 2 * 16 * 256], BF16, tag="bcd")
            bcdv = bcd[:].rearrange("p (a n t) -> p a n t", a=2, n=16)
            nc.sync.dma_start(out=bcdv[:, 0], in_=Brow)
            nc.sync.dma_start(out=bcdv[:, 1], in_=Crow)
            bcr = mq.tile([128, 2, 16, 256], BF16, tag="bcr")
            src = bass.AP(
                tensor=bcd.tensor, offset=bcd.offset,
                ap=[[0, 128]] + [list(p) for p in bcd.ap[1:]],
            )
            nc.sync.dma_start(out=bcr.rearrange("p a n t -> p (a n t)"), in_=src)

            y2t = mq.tile([128, ndc, 256], BF16, tag="y2t")
            for dc in range(ndc):
                u = mq.tile([128, 256], BF16, tag="u")
                nc.vector.tensor_tensor(
                    out=u, in0=dt_sb[:, dc, :], in1=xcp[:, dc, :], op=ALU.mult
                )
                dA = mq.tile([128, 16, 256], F32, tag="dA")
                for n in range(16):
                    nc.scalar.activation(
                        dA[:, n, :], dt_sb[:, dc, :], AF.Exp,
                        scale=A_t[:, dc, n : n + 1],
                    )
                nc.vector.memset(dA[:, :, 0:1], 0.0)
                dBx = mq.tile([128, 16, 256], BF16, tag="dBx")
                ub = bass.AP(
                    tensor=u.tensor, offset=u.offset,
                    ap=[list(u.ap[0]), [0, 16], list(u.ap[1])],
                )
                nc.vector.tensor_tensor(out=dBx, in0=bcr[:, 0], in1=ub, op=ALU.mult)
                hs = mq.tile([128, 16, 256], BF16, tag="hs")
                nc.vector.tensor_tensor_scan(
                    out=hs.rearrange("p n t -> p (n t)"),
                    data0=dA.rearrange("p n t -> p (n t)"),
                    data1=dBx.rearrange("p n t -> p (n t)"),
                    initial=0.0, op0=ALU.mult, op1=ALU.add,
                )
                nc.vector.tensor_tensor(out=hs, in0=hs, in1=bcr[:, 1], op=ALU.mult)
                for half in (8, 4, 2, 1):
                    nc.vector.tensor_tensor(
                        out=hs[:, 0:half], in0=hs[:, 0:half],
                        in1=hs[:, half : 2 * half], op=ALU.add,
                    )
                t1 = mq.tile([128, 256], BF16, tag="t1")
                nc.vector.scalar_tensor_tensor(
                    out=t1, in0=xcp[:, dc, :],
                    scalar=cons_t[:, 32 + dc : 33 + dc],
                    in1=hs[:, 0, :], op0=ALU.mult, op1=ALU.add,
                )
                nc.vector.tensor_tensor(
                    out=y2t[:, dc, :], in0=t1, in1=zs[:, dc, :], op=ALU.mult
                )

            if DEBUG:
                pfx = "e" if en_mode else ""
                nc.sync.dma_start(
                    out=dbg["d_" + ("excp" if en_mode else "xcp")][:],
                    in_=xcp.rearrange("p a t -> p (a t)"),
                )
                nc.sync.dma_start(
                    out=dbg["d_" + pfx + "dt"][:],
                    in_=dt_sb.rearrange("p a t -> p (a t)"),
                )
                nc.sync.dma_start(
                    out=dbg["d_" + pfx + "y2"][:],
                    in_=y2t.rearrange("p a t -> p (a t)"),
                )

            opv = opT_d[:].rearrange("p (k m c) -> p k m c", k=ndc, m=out_mc)
            o_dt = BF16
            o_sb = mq.tile([128, out_mc, 256], o_dt, tag="osb")
            for mg in range(out_mc // 4):
                pst = [psum.tile([128, 256], F32, tag="mm", name=f"pst{_i}") for _i in range(4)]
                for kc in range(ndc):
                    wto = mw.tile([128, 4, 128], BF16, tag="wto")
                    nc.sync.dma_start(out=wto, in_=opv[:, kc, 4 * mg : 4 * mg + 4, :])
                    for i in range(4):
                        nc.tensor.matmul(
                            pst[i], wto[:, i, :], y2t[:, kc, :],
                            start=(kc == 0), stop=(kc == ndc - 1),
                        )
                for i in range(4):
                    nc.scalar.activation(o_sb[:, 4 * mg + i, :], pst[i], AF.Copy)

            if en_mode:
                oin = dram.tile([128, out_mc * 256], BF16, tag="oin")
                oout = dram.tile([128, out_mc * 256], BF16, tag="oout")
                nc.sync.dma_start(out=oin, in_=o_sb.rearrange("p a t -> p (a t)"))
                nc.gpsimd.collective_compute(
                    "AllReduce", ALU.add, replica_groups=RG,
                    ins=[oin.opt()], outs=[oout.opt()],
                )
                nc.sync.dma_start(out=o_sb.rearrange("p a t -> p (a t)"), in_=oout[:])
            if DEBUG:
                nc.sync.dma_start(
                    out=dbg["d_eo" if en_mode else "d_oco"][:],
                    in_=o_sb.rearrange("p a t -> p (a t)"),
                )

            D_ = out_mc * 128
            sq = mq.tile([128, out_mc, 256], BF16, tag="sq")
            nc.vector.tensor_tensor(out=sq, in0=o_sb, in1=o_sb, op=ALU.mult)
            ones_t = onesb_t
            pss_ = psum.tile([1, 256], F32, tag="mm")
            psq_ = psum.tile([1, 256], F32, tag="mm")
            for kc in range(out_mc):
                nc.tensor.matmul(
                    pss_, ones_t, o_sb[:, kc, :],
                    start=(kc == 0), stop=(kc == out_mc - 1),
                )
            for kc in range(out_mc):
                nc.tensor.matmul(
                    psq_, ones_t, sq[:, kc, :],
                    start=(kc == 0), stop=(kc == out_mc - 1),
                )
            mrs = mq.tile([1, 2, 256], F32, tag="mrs")
            nc.vector.tensor_scalar_mul(mrs[:, 0, :], pss_, 1.0 / D_)
            msq = mq.tile([1, 256], F32, tag="msq")
            nc.vector.tensor_scalar_mul(msq, psq_, 1.0 / D_)
            var = mq.tile([1, 256], F32, tag="var")
            nc.vector.tensor_tensor(
                out=var, in0=mrs[:, 0, :], in1=mrs[:, 0, :], op=ALU.mult
            )
            nc.vector.tensor_tensor(out=var, in0=msq, in1=var, op=ALU.subtract)
            nc.vector.tensor_scalar_add(var, var, 1e-6)
            std = mq.tile([1, 256], F32, tag="std")
            nc.scalar.activation(std, var, AF.Sqrt)
            nc.vector.reciprocal(mrs[:, 1, :], std)
            mrd = dram.tile([1, 512], F32, tag="mrd")
            nc.sync.dma_start(out=mrd, in_=mrs.rearrange("p a t -> p (a t)"))
            mrb = mq.tile([128, 2, 256], F32, tag="mrb")
            src2 = bass.AP(
                tensor=mrd.tensor, offset=mrd.offset,
                ap=[[0, 128]] + [list(p) for p in mrd.ap[1:]],
            )
            nc.sync.dma_start(out=mrb.rearrange("p a t -> p (a t)"), in_=src2)
            g_sl, b_sl = lnslots
            for mc in range(out_mc):
                xn = mq.tile([128, 256], F32, tag="xn")
                nc.vector.tensor_tensor(
                    out=xn, in0=o_sb[:, mc, :], in1=mrb[:, 0, :], op=ALU.subtract
                )
                nc.vector.tensor_tensor(out=xn, in0=xn, in1=mrb[:, 1, :], op=ALU.mult)
                gb = mq.tile([128, 256], BF16, tag="gb")
                nc.vector.tensor_scalar_add(
                    gb, resid[:, mc, :], cons_t[:, b_sl + mc : b_sl + mc + 1]
                )
                nc.vector.scalar_tensor_tensor(
                    out=x_out[:, mc, :], in0=xn,
                    scalar=cons_t[:, g_sl + mc : g_sl + mc + 1],
                    in1=gb, op0=ALU.mult, op1=ALU.add,
                )
            close(pref + "_mw", pref + "_mq", pref + "_mp")

        close("resq", "resw", "carryA")

        # ---- co-mamba (local) ----
        x1 = carry.tile([128, 8, 256], BF16)
        mamba("co", 8, g1, ipT, cocw, cocons, coxpT, codtT, coA,
              coopT, 8, (48, 56), g1, x1, False)
        if DEBUG:
            nc.sync.dma_start(out=dbg["d_x1"][:], in_=x1.rearrange("p a t -> p (a t)"))
        close("carryB")

        # ======================= PHASE D: En =======================
        ag1i = dram.tile([1024, 256], BF16, tag="ag1i")
        ag1o = dram.tile([2048, 256], BF16, tag="ag1o")
        nc.sync.dma_start(out=ag1i[:].rearrange("(a p) t -> p a t", p=128), in_=x1)
        nc.gpsimd.collective_compute(
            "AllGather", ALU.bypass, replica_groups=RG,
            ins=[ag1i.opt()], outs=[ag1o.opt()],
        )
        xfullpad = carry.tile([128, 16, 258], BF16)
        nc.vector.memset(xfullpad[:, :, 0:1], 0.0)
        nc.vector.memset(xfullpad[:, :, 257:258], 0.0)
        nc.sync.dma_start(
            out=xfullpad[:, :, 1:257],
            in_=ag1o[:].rearrange("(a p) t -> p a t", p=128),
        )

        enrw = pool("enrw", 2)
        enbn_t = enrw.tile([128, 8, 2], F32, tag="enbn")
        nc.sync.dma_start(out=enbn_t, in_=enbnsb[:].rearrange("p (a b) -> p a b", b=2))
        eh = enrw.tile([128, 8, 256], BF16, tag="eh")
        env = enT[:].rearrange("p (m k t c) -> p m k t c", m=8, k=16, t=3)
        for mc in range(8):
            pse = psum.tile([128, 256], F32, tag="mm")
            ewt = enrw.tile([128, 16, 3, 128], BF16, tag="ewt")
            nc.sync.dma_start(out=ewt, in_=env[:, mc])
            for tap in range(3):
                for kc in range(16):
                    nc.tensor.matmul(
                        pse, ewt[:, kc, tap, :], xfullpad[:, kc, tap : tap + 256],
                        start=(tap == 0 and kc == 0), stop=(tap == 2 and kc == 15),
                    )
            tse = enrw.tile([128, 256], F32, tag="tse")
            nc.scalar.activation(
                tse, pse, AF.Relu, bias=enbn_t[:, mc, 1:2], scale=enbn_t[:, mc, 0:1]
            )
            nc.vector.tensor_tensor(
                out=eh[:, mc, :], in0=tse, in1=x1[:, mc, :], op=ALU.add
            )
        ag2i = dram.tile([1024, 256], BF16, tag="ag2i")
        ag2o = dram.tile([2048, 256], BF16, tag="ag2o")
        nc.sync.dma_start(out=ag2i[:].rearrange("(a p) t -> p a t", p=128), in_=eh)
        close("enrw")
        nc.gpsimd.collective_compute(
            "AllGather", ALU.bypass, replica_groups=RG,
            ins=[ag2i.opt()], outs=[ag2o.opt()],
        )
        xe = carry.tile([128, 16, 256], BF16)
        nc.sync.dma_start(out=xe, in_=ag2o[:].rearrange("(a p) t -> p a t", p=128))
        if DEBUG:
            nc.sync.dma_start(out=dbg["d_xe"][:], in_=xe.rearrange("p a t -> p (a t)"))

        xf = carry.tile([128, 16, 256], BF16)
        mamba("en", 16, xe, eipT, encw, encons, expT, edtT0, enA,
              eopT, 16, (48, 64), xe, xf, True)
        if DEBUG:
            nc.sync.dma_start(out=dbg["d_xf"][:], in_=xf.rearrange("p a t -> p (a t)"))

        clst = carry.tile([128, 16, 7], BF16)
        nc.sync.dma_start(out=clst, in_=clsT[:].rearrange("p (k m) -> p k m", k=16))
        clsb_t = carry.tile([7, 1], F32)
        nc.sync.dma_start(out=clsb_t, in_=clsb[:])
        psl = psum.tile([7, 1], F32, tag="mm")
        for kc in range(16):
            nc.tensor.matmul(
                psl, clst[:, kc, :], xf[:, kc, 255:256],
                start=(kc == 0), stop=(kc == 15),
            )
        lg = carry.tile([7, 1], F32)
        nc.vector.tensor_tensor(out=lg, in0=psl, in1=clsb_t, op=ALU.add)
        nc.sync.dma_start(out=logits[:], in_=lg)

        for p in reversed(list(ctxs.values())):
            p.__exit__(None, None, None)

    nc.finalize()
    return nc


# ======================= host-side packing =======================


def _bf(x):
    return np.ascontiguousarray(np.asarray(x, np.float32)).astype(BF)


def _f32(x):
    return np.ascontiguousarray(np.asarray(x, np.float32))


def _chunkT(w, kdim_first=True):
    """w [M, K] -> lhsT layout [K, M] as [128*(K/128) rows]."""
    return np.ascontiguousarray(np.asarray(w, np.float32).T)


def _pack_mk(wT, nkc, nmc):
    """wT [K, M] -> [128, nkc*nmc*128]: out[p, kc, mc, m] = wT[kc*128+p, mc*128+m]."""
    K, M = wT.shape
    assert K == nkc * 128 and M == nmc * 128
    r = wT.reshape(nkc, 128, nmc, 128).transpose(1, 0, 2, 3)
    return np.ascontiguousarray(r.reshape(128, nkc * nmc * 128))


def _cnn_pack(cp):
    c1 = np.asarray(cp["c1w"], np.float32)  # [64,4,5,5]
    w1T = c1.transpose(2, 3, 1, 0).reshape(100, 64)  # rows (dy,dx,c)
    c2 = np.asarray(cp["c2w"], np.float32)  # [128,64,3,3]
    w2T = c2.transpose(1, 2, 3, 0).reshape(64, 9, 128).transpose(0, 1, 2)
    w2T = np.ascontiguousarray(c2.transpose(1, 2, 3, 0).reshape(64, 9 * 128))
    # careful: want w2T[k, tap, m]: c2.transpose(1,2,3,0) -> [64, 3, 3, 128]
    c3 = np.asarray(cp["c3w"], np.float32)  # [256,128,3,3]
    w3 = c3.transpose(1, 2, 3, 0).reshape(128, 9, 256)  # [k, tap, M]
    w3T = w3.reshape(128, 9, 2, 128)
    c4 = np.asarray(cp["c4w"], np.float32)[:, :, 0, 0]  # [64,256]
    w4T = c4.T.reshape(2, 128, 64).transpose(1, 0, 2)  # [128, 2, 64]
    fw = np.asarray(cp["fcw"], np.float32)  # [512, 1024]
    fcT = fw.T.reshape(8, 128, 4, 128).transpose(1, 0, 2, 3)  # [128, 8, 4, 128]
    cnnb = np.zeros((128, 9), np.float32)
    cnnb[0:64, 0] = np.asarray(cp["c1b"], np.float32)
    cnnb[:, 1] = np.asarray(cp["c2b"], np.float32)
    b3 = np.asarray(cp["c3b"], np.float32)
    cnnb[:, 2] = b3[0:128]
    cnnb[:, 3] = b3[128:256]
    cnnb[0:64, 4] = np.asarray(cp["c4b"], np.float32)
    bf_ = np.asarray(cp["fcb"], np.float32)
    for mc in range(4):
        cnnb[:, 5 + mc] = bf_[mc * 128 : (mc + 1) * 128]
    return (
        _bf(w1T), _bf(w2T), _bf(w3T.reshape(128, -1)), _bf(w4T.reshape(128, -1)),
        _bf(fcT.reshape(128, -1)), cnnb,
    )


def _bn_fold(bn, rows=None):
    g = np.asarray(bn["g"], np.float32)
    b = np.asarray(bn["b"], np.float32)
    m = np.asarray(bn["mean"], np.float32)
    v = np.asarray(bn["var"], np.float32)
    sc = g / np.sqrt(v + 1e-5)
    bi = b - m * sc
    if rows is not None:
        sc, bi = sc[rows], bi[rows]
    out = np.stack([sc, bi], -1).reshape(-1, 128, 2).transpose(1, 0, 2)
    return _f32(out.reshape(128, -1))


def _mamba_pack(mpz, s, en):
    """Pack one mamba's params. en: shard d_inner rows s*2048:(s+1)*2048."""
    ip = np.asarray(mpz["in_proj"], np.float32)  # [2*di, d]
    di = ip.shape[0] // 2
    d = ip.shape[1]
    if en:
        sh = slice(s * 2048, (s + 1) * 2048)
        rows = np.concatenate([ip[0:di][sh], ip[di : 2 * di][sh]], 0)  # [4096, d]
    else:
        rows = ip  # [4096, 1024]
    nkc = d // 128
    ipT = _pack_mk(rows.T, nkc, 32)  # [128, nkc*32*128]

    dsl = slice(s * 2048, (s + 1) * 2048) if en else slice(0, di)
    cw = np.asarray(mpz["conv_w"], np.float32)[dsl]  # [2048, 4]
    cwp = cw.reshape(16, 128, 4).transpose(1, 0, 2).reshape(128, 64)
    cb = np.asarray(mpz["conv_b"], np.float32)[dsl]
    dtb = np.asarray(mpz["dt_b"], np.float32)[dsl]
    Dv = np.asarray(mpz["D"], np.float32)[dsl]
    A = -np.exp(np.asarray(mpz["A_log"], np.float32))[dsl]  # [2048, 16]
    Ap = A.reshape(16, 128, 16).transpose(1, 0, 2).reshape(128, 256)

    xp = np.asarray(mpz["x_proj"], np.float32)  # [dtr+32, di]
    if en:
        xps = xp[:, s * 2048 : (s + 1) * 2048].T  # [2048, 160]
        xpad = np.zeros((2048, 256), np.float32)
        xpad[:, 0:128] = xps[:, 0:128]
        xpad[:, 128:160] = xps[:, 128:160]
        xpT = _f32(
            xpad.reshape(16, 128, 2, 128).transpose(1, 0, 2, 3).reshape(128, -1)
        )
    else:
        xpT = _f32(xp.T.reshape(16, 128, 96).transpose(1, 0, 2).reshape(128, -1))

    dw = np.asarray(mpz["dt_w"], np.float32)[dsl]  # [d_inner_sh, dtr]
    dtr = dw.shape[1]
    dt0 = _f32(dw.T.reshape(dtr, 16, 128).reshape(dtr, -1))
    dt1 = None

    op = np.asarray(mpz["out_proj"], np.float32)  # [d, di]
    opc = op[:, s * 2048 : (s + 1) * 2048] if en else op
    opT = _pack_mk(opc.T, 16, d // 128)

    def colpack(v):
        return v.reshape(-1, 128).T  # [128, nch]

    ncons = 80 if en else 64
    cons = np.zeros((128, ncons), np.float32)
    cons[:, 0:16] = colpack(cb)
    cons[:, 16:32] = colpack(dtb)
    cons[:, 32:48] = colpack(Dv)
    return dict(
        ipT=_bf(ipT), cw=_f32(cwp), xpT=_bf(xpT), dt0=_bf(dt0),
        dt1=(_bf(dt1) if dt1 is not None else None), opT=_bf(opT),
        A=_f32(Ap), cons=cons,
    )


def _prep(g, r, params):
    g = np.asarray(g, np.float32)
    r = np.asarray(r, np.float32)
    P = {k: v for k, v in params.items()}
    in_maps = []
    for c in range(8):
        s, b = c // 4, c % 4
        x = (g if s == 0 else r)[b]  # [256, 4, 8, 9]
        xp = np.zeros((4, T, 12, 13), np.float32)
        xp[:, :, 2:10, 2:11] = x.transpose(1, 0, 2, 3)
        imc = np.zeros((25, 4, T, 8, 9), np.float32)
        for tap in range(25):
            dy, dx = tap // 5, tap % 5
            imc[tap] = xp[:, :, dy : dy + 8, dx : dx + 9]
        m = dict(gimc=_bf(imc.reshape(100, T * 72)))
        cp = P["cnn_g"] if s == 0 else P["cnn_r"]
        (m["w1T"], m["w2T"], m["w3T"], m["w4T"], m["fcT"], m["cnnb"]) = _cnn_pack(cp)
        m["onesb"] = np.ones((128, 1), BF)
        m["onesf"] = np.ones((128, 1), np.float32)

        pr = P["co_res"]
        ckey, skey, bnkey = ("e_conv", "e_skip", "e_bn") if s == 0 else (
            "g_conv", "g_skip", "g_bn")
        ec = np.asarray(pr[ckey], np.float32)  # [1024, 512, 3]
        ecT = ec.transpose(1, 2, 0).reshape(4, 128, 3, 8, 128)
        # want [128, kc, tap, mc, m]: transpose(1,0,2,3,4)
        m["ecT"] = _bf(ecT.transpose(1, 0, 2, 3, 4).reshape(128, -1))
        es = np.asarray(pr[skey], np.float32)[:, :, 0]  # [1024, 512]
        m["esT"] = _bf(
            es.T.reshape(4, 128, 8, 128).transpose(1, 0, 2, 3).reshape(128, -1)
        )
        m["ebnsb"] = _bn_fold(pr[bnkey])

        mco = P["co_mamba_g"] if s == 0 else P["co_mamba_r"]
        pk = _mamba_pack(mco, s, False)
        m["ipT"], m["cocw"], m["coxpT"] = pk["ipT"], pk["cw"], pk["xpT"]
        m["codtT"], m["coopT"], m["coA"] = pk["dt0"], pk["opT"], pk["A"]
        cons = pk["cons"]
        lng = np.asarray(P["co_ln1_g" if s == 0 else "co_ln2_g"], np.float32)
        lnb = np.asarray(P["co_ln1_b" if s == 0 else "co_ln2_b"], np.float32)
        cons[:, 48:56] = lng.reshape(8, 128).T
        cons[:, 56:64] = lnb.reshape(8, 128).T
        m["cocons"] = cons

        en = np.asarray(P["en_conv"], np.float32)  # [2048, 2048, 3]
        rows = slice(s * 1024, (s + 1) * 1024)
        enc = en[rows].transpose(1, 2, 0)  # [2048k, 3, 1024m]
        enc = enc.reshape(16, 128, 3, 8, 128)  # [kc, p, tap, mc, m]
        m["enT"] = _bf(
            enc.transpose(1, 3, 0, 2, 4).reshape(128, -1)  # [p, mc, kc, tap, m]
        )
        m["enbnsb"] = _bn_fold(P["en_bn"], rows)

        pe = _mamba_pack(P["en_mamba"], s, True)
        m["eipT"], m["encw"], m["expT"] = pe["ipT"], pe["cw"], pe["xpT"]
        m["edtT0"] = pe["dt0"]
        m["eopT"], m["enA"] = pe["opT"], pe["A"]
        econs = pe["cons"]
        econs[:, 48:64] = np.asarray(P["en_ln_g"], np.float32).reshape(16, 128).T
        econs[:, 64:80] = np.asarray(P["en_ln_b"], np.float32).reshape(16, 128).T
        m["encons"] = econs

        cw_ = np.asarray(P["cls_w"], np.float32)  # [7, 2048]
        m["clsT"] = _bf(cw_.T.reshape(16, 128, 7).transpose(1, 0, 2).reshape(128, -1))
        m["clsb"] = _f32(np.asarray(P["cls_b"], np.float32).reshape(7, 1))
        in_maps.append(m)
    return in_maps


def kernel(g, r, params):
    from concourse.bass_utils import run_bass_kernel_spmd

    if "nc" not in _CACHE:
        _CACHE["nc"] = _build()
    in_maps = _prep(g, r, params)
    trace = int(os.environ.get("BMV_TRACE", "0"))
    res = run_bass_kernel_spmd(
        _CACHE["nc"], in_maps, core_ids=list(range(8)), trace=bool(trace)
    )
    _CACHE["last"] = res
    out = np.stack(
        [np.asarray(res.results[b]["logits"], np.float32)[:, 0] for b in range(4)]
    )
    return out.astype(np.float32)
